# revision 50
# baseline (speedup 1.0000x reference)
"""GQA attention block (RoPE + causal attention + output proj) on 8 TRN2 NeuronCores.

Sharding: batch (B=2) x kv-head-group (KVH=4) -> 8 cores.
Core c handles batch b=c//4, kv group g=c%4 (q heads 4g..4g+3, kv head g).

No collective: wo is ROW-split (each core keeps the 256 wo rows that its own
4 heads feed), so every core computes a full-width [HID, S] PARTIAL output
projection from its local head outputs; the host sums the 4 partials per
batch (bf16 partials, fp32 host accumulate). This removes the AllGather +
DRAM bounce + gather DMAs of the column-split formulation and lets each
span's output projection pipeline right behind its attention.

All matmuls run in bf16 (fp32 PSUM accumulation). Layouts are transposed
([feature, token]) so Q/K/V projections, scores (computed as S^T =
K-stationary), and PV all feed the TensorEngine without transposes; softmax
runs without max-subtraction (logits are provably small for this problem's
scale).

Dependency-granularity note: Tile tracks deps per-tile, so x/q/k live in
per-half tiles (attention on span J only waits for the half it reads) and
the x DMA is split into 128KB pieces across all 16 queues.

PSUM budget: scores/oproj/bcast pool 2x2 banks + per-pair PV accumulators
2x2 banks = 8 banks.
"""

import sys

if "/opt/trn_rl_repo" not in sys.path:
    sys.path.insert(0, "/opt/trn_rl_repo")

import numpy as np
import ml_dtypes

import concourse.bass as bass
import concourse.mybir as mybir
import concourse.tile as tile
from concourse import bacc
from concourse.bass_utils import run_bass_kernel_spmd

BF16 = ml_dtypes.bfloat16

B, S, HID = 2, 2048, 1024
H, KVH, D = 16, 4, 64
G = H // KVH
N_CORES = 8
SPAN = 512
NSPAN = S // SPAN  # 4
NCH = HID // 128  # 8 contraction chunks
NKT = S // 128  # 16 k-tiles
HS = S // 2
F32 = mybir.dt.float32
BF = mybir.dt.bfloat16

TRACE = False
_CACHED = {}


def _build_nc():
    nc = bacc.Bacc("TRN2", target_bir_lowering=False, debug=False, num_devices=N_CORES)

    xT = nc.dram_tensor("xT", [HID, S], BF, kind="ExternalInput")
    wq = nc.dram_tensor("wq", [HID, 256], BF, kind="ExternalInput")
    wkv = nc.dram_tensor("wkv", [HID, 128], BF, kind="ExternalInput")
    woR = nc.dram_tensor("woR", [256, HID], BF, kind="ExternalInput")
    c2 = nc.dram_tensor("c2", [128, S], BF, kind="ExternalInput")
    s2 = nc.dram_tensor("s2", [128, S], BF, kind="ExternalInput")
    c1 = nc.dram_tensor("c1", [64, S], BF, kind="ExternalInput")
    s1 = nc.dram_tensor("s1", [64, S], BF, kind="ExternalInput")
    dmask = nc.dram_tensor("dmask", [128, 128], BF, kind="ExternalInput")
    out = nc.dram_tensor("out", [HID, S], BF, kind="ExternalOutput")

    EXP = mybir.ActivationFunctionType.Exp

    with tile.TileContext(nc) as tc:
        with (
            tc.tile_pool(name="main", bufs=1) as main,
            # created before ropep so they sit below it in SBUF: the
            # attention-phase tiles must not reuse ropep's region, or the
            # first exp stalls on a WAR hazard behind the half-1 RoPE reads
            tc.tile_pool(name="pp", bufs=9) as pp,
            tc.tile_pool(name="work", bufs=2) as work,
        ):
            # ---- persistent SBUF (per-half tiles where attention reads) ----
            xTh = [
                [main.tile([128, HS], BF, name=f"xT{k}_{hf}") for hf in range(2)]
                for k in range(NCH)
            ]
            wq_sb = [main.tile([128, 256], BF, name=f"wq{k}") for k in range(NCH)]
            wkv_sb = [main.tile([128, 128], BF, name=f"wkv{k}") for k in range(NCH)]
            wo_sb = [main.tile([128, HID], BF, name=f"wo{c}") for c in range(2)]
            c2_sb = main.tile([128, S], BF, name="c2_sb")
            s2_sb = main.tile([128, S], BF, name="s2_sb")
            c1_sb = main.tile([64, S], BF, name="c1_sb")
            s1_sb = main.tile([64, S], BF, name="s1_sb")
            dmask_sb = main.tile([128, 128], BF, name="dmask_sb")
            qTh = [
                [main.tile([128, HS], BF, name=f"qT{p}_{hf}") for hf in range(2)]
                for p in range(2)
            ]
            kTh = [main.tile([128, HS], BF, name=f"kT_{hf}") for hf in range(2)]
            vTh = [main.tile([64, HS], BF, name=f"vT_{hf}") for hf in range(2)]
            vaug_sb = main.tile([128, NKT, 128], BF, name="vaug_sb")
            ones_sb = main.tile([1, 64], BF, name="ones_sb")
            dums_sb = main.tile([128, SPAN], BF, name="dums_sb")

            # kv weights (scalar DGE) + first-half x pieces (desc-gen spread
            # across sync/gpsimd so the serial ~610ns/DMA descriptor
            # generation doesn't gate the first projection matmul)
            for k in range(NCH):
                nc.scalar.dma_start(wkv_sb[k][:], wkv[128 * k : 128 * k + 128, :])
            eng0 = [nc.sync, nc.gpsimd]
            for pc in range(2):
                for k in range(NCH):
                    eng0[k % 2].dma_start(
                        xTh[k][0][:, SPAN * pc : SPAN * (pc + 1)],
                        xT[128 * k : 128 * k + 128, SPAN * pc : SPAN * (pc + 1)],
                    )
            for k in range(NCH):
                nc.scalar.dma_start(wq_sb[k][:], wq[128 * k : 128 * k + 128, :])
            nc.sync.dma_start(c1_sb[:], c1[:])
            nc.sync.dma_start(s1_sb[:], s1[:])
            nc.sync.dma_start(c2_sb[:], c2[:])
            nc.sync.dma_start(s2_sb[:], s2[:])
            nc.sync.dma_start(dmask_sb[:], dmask[:])
            for pc in range(2):
                for k in range(NCH):
                    eng0[k % 2].dma_start(
                        xTh[k][1][:, SPAN * pc : SPAN * (pc + 1)],
                        xT[128 * k : 128 * k + 128, HS + SPAN * pc : HS + SPAN * (pc + 1)],
                    )
            for c in range(2):
                nc.scalar.dma_start(wo_sb[c][:], woR[128 * c : 128 * c + 128, :])
            nc.vector.memset(ones_sb[:], 1.0)
            nc.vector.memset(dums_sb[:], 0.0)
            # cols 64:128 of each vaug k-tile: col 64 = ones row (softmax
            # denominator accumulator), 65:128 zero-padding so the PV lhsT is
            # a full 128-column tile (M=65 hits a slow PE path)
            nc.vector.memset(vaug_sb[:, :, 64:128], 0.0)
            nc.vector.memset(vaug_sb[:, :, 64:65], 1.0)

            # ---- phase 1: projections (transposed layout) + RoPE; KV first so
            # the V-transpose can run while the Q projections are still going ----
            with (
                tc.tile_pool(name="psA", bufs=3, space="PSUM") as psA,
                tc.tile_pool(name="psD", bufs=1, space="PSUM") as psD,
                tc.tile_pool(name="ropep", bufs=2) as ropep,
            ):
                # dummy matmuls interleaved with the DMA-paced first chunk
                # matmuls: they fill the arrival gaps so the PE p-state ramp
                # (0.65 -> 1.2 -> 2.4 GHz after 3us continuous) is not reset
                dump = psD.tile([128, SPAN], F32, tag="dum", name="dump")

                def dummy(n):
                    for _ in range(n):
                        nc.tensor.matmul(
                            dump[:], dums_sb[:, 0:128], dums_sb[:], start=True,
                            stop=True,
                        )

                for hf in range(2):
                    f0 = HS * hf
                    kvp = psA.tile([128, HS], F32, tag="qkv", name=f"kvp{hf}")
                    for sp in range(2):
                        for k in range(NCH):
                            if hf == 0 and sp == 0:
                                dummy(2)
                            nc.tensor.matmul(
                                kvp[:, SPAN * sp : SPAN * (sp + 1)],
                                wkv_sb[k][:],
                                xTh[k][hf][:, SPAN * sp : SPAN * (sp + 1)],
                                start=(k == 0),
                                stop=(k == NCH - 1),
                            )
                    kb = ropep.tile([64, HS], BF, tag="kb", name=f"kb{hf}")
                    nc.scalar.copy(kb[:], kvp[0:64, :])
                    nc.scalar.copy(vTh[hf][:], kvp[64:128, :])
                    tcosk = ropep.tile([64, HS], BF, tag="tcos", name=f"tcosk{hf}")
                    tsink = ropep.tile([64, HS], BF, tag="tsin", name=f"tsink{hf}")
                    nc.vector.tensor_mul(tcosk[:], kb[:], c1_sb[:, f0 : f0 + HS])
                    for dst, src in ((0, 32), (32, 0)):
                        nc.vector.tensor_mul(
                            tsink[dst : dst + 32, :],
                            kb[src : src + 32, :],
                            s1_sb[src : src + 32, f0 : f0 + HS],
                        )
                    nc.vector.tensor_add(kTh[hf][0:64, :], tcosk[:], tsink[:])
                    nc.vector.tensor_copy(kTh[hf][64:128, :], kTh[hf][0:64, :])
                    # V transpose to [token, d] for this half via DMA XBAR
                    # (keeps the PE stream pure matmuls)
                    for t in range(8 * hf, 8 * hf + 8):
                        nc.sync.dma_start_transpose(
                            vaug_sb[:, t, 0:64],
                            vTh[hf][:, 128 * (t - 8 * hf) : 128 * (t - 8 * hf + 1)],
                        )
                    for p in range(2):
                        qp = psA.tile([128, HS], F32, tag="qkv", name=f"qp{p}_{hf}")
                        for sp in range(2):
                            for k in range(NCH):
                                nc.tensor.matmul(
                                    qp[:, SPAN * sp : SPAN * (sp + 1)],
                                    wq_sb[k][:, 128 * p : 128 * (p + 1)],
                                    xTh[k][hf][:, SPAN * sp : SPAN * (sp + 1)],
                                    start=(k == 0),
                                    stop=(k == NCH - 1),
                                )
                        qb = ropep.tile([128, HS], BF, tag="qb", name=f"qb{p}{hf}")
                        nc.scalar.copy(qb[:], qp[:])
                        tcos = ropep.tile([128, HS], BF, tag="tcos", name=f"tc{p}{hf}")
                        tsin = ropep.tile([128, HS], BF, tag="tsin", name=f"ts{p}{hf}")
                        nc.vector.tensor_mul(tcos[:], qb[:], c2_sb[:, f0 : f0 + HS])
                        for dst, src in ((0, 32), (32, 0), (64, 96), (96, 64)):
                            nc.vector.tensor_mul(
                                tsin[dst : dst + 32, :],
                                qb[src : src + 32, :],
                                s2_sb[src : src + 32, f0 : f0 + HS],
                            )
                        nc.vector.tensor_add(qTh[p][hf][:], tcos[:], tsin[:])

            # ---- phase 2: attention spans + local (partial) output projection ----
            with (
                tc.tile_pool(name="psS", bufs=2, space="PSUM") as psS,
                tc.tile_pool(name="psO", bufs=1, space="PSUM") as psO,
                tc.tile_pool(name="psP", bufs=2, space="PSUM") as psP,
            ):

                def pair_attention(J, pr):
                    qh = SPAN * (J % 2)
                    nkt_j = 4 * (J + 1)
                    src = qTh[pr][J // 2]
                    opsum = psO.tile(
                        [128, 2, SPAN], F32, tag="o", name=f"opsum{J}_{pr}"
                    )
                    pv_queue = []

                    def emit_pv(j, pt, off):
                        for hh in range(2):
                            nc.tensor.matmul(
                                opsum[:, hh, off:SPAN],
                                vaug_sb[:, j, :],
                                pt[:, hh, off:SPAN],
                                start=(j == 0),
                                stop=(j == nkt_j - 1),
                            )

                    for jb in range(0, nkt_j, 2):
                        batch = []
                        for j in range(jb, min(jb + 2, nkt_j)):
                            jj = j - 4 * J
                            off = 128 * jj if jj > 0 else 0
                            kt = kTh[j // 8]
                            kc = 128 * (j % 8)
                            sps = psS.tile(
                                [128, 2, SPAN], F32, tag="s", name=f"s{J}_{j}_{pr}"
                            )
                            pt = pp.tile(
                                [128, 2, SPAN], BF, tag="p", name=f"p{J}_{j}_{pr}"
                            )
                            nc.tensor.matmul(
                                sps[:, 0, off:SPAN],
                                kt[0:64, kc : kc + 128],
                                src[0:64, qh + off : qh + SPAN],
                                start=True,
                                stop=True,
                            )
                            nc.tensor.matmul(
                                sps[:, 1, off:SPAN],
                                kt[64:128, kc : kc + 128],
                                src[64:128, qh + off : qh + SPAN],
                                start=True,
                                stop=True,
                            )
                            batch.append((j, sps, pt, off))
                        for j, sps, pt, off in batch:
                            nc.scalar.activation(
                                pt[:, :, off:SPAN], sps[:, :, off:SPAN], EXP
                            )
                            jj = j - 4 * J
                            if jj >= 0:
                                # masks on the Pool engine: DVE's in-order
                                # queue would stall them behind RoPE/norm work
                                for hh in range(2):
                                    nc.gpsimd.tensor_mul(
                                        pt[:, hh, off : off + 128],
                                        pt[:, hh, off : off + 128],
                                        dmask_sb[:],
                                    )
                            pv_queue.append((j, pt, off))
                        while len(pv_queue) > 4:
                            emit_pv(*pv_queue.pop(0))
                            emit_pv(*pv_queue.pop(0))
                    for args in pv_queue:
                        emit_pv(*args)

                    # normalization: denom -> bcast -> 1/x -> scale into hout
                    dsb = work.tile([1, 2 * SPAN], BF, tag="dsb", name=f"dsb{J}_{pr}")
                    nc.vector.tensor_copy(dsb[:], opsum[64:65, :, :])
                    hout = work.tile(
                        [128, SPAN], BF, tag=f"h{pr}", bufs=2, name=f"h{J}_{pr}"
                    )
                    for hh in range(2):
                        bc = psP.tile([64, SPAN], F32, tag="po", name=f"bc{J}_{pr}{hh}")
                        nc.tensor.matmul(
                            bc[:],
                            ones_sb[:],
                            dsb[0:1, SPAN * hh : SPAN * (hh + 1)],
                            start=True,
                            stop=True,
                        )
                        rec = work.tile(
                            [64, SPAN], F32, tag="rec", name=f"rec{J}_{pr}{hh}"
                        )
                        nc.vector.reciprocal_approx_fast(rec[:], bc[:])
                        nc.vector.tensor_mul(
                            hout[64 * hh : 64 * (hh + 1), :],
                            opsum[0:64, hh, :],
                            rec[:],
                        )
                    return hout

                def emit_oproj(J, h0, h1):
                    q0 = SPAN * J
                    engs = [nc.sync, nc.scalar]
                    for m in range(8):
                        po = psP.tile([128, SPAN], F32, tag="po", name=f"po{J}_{m}")
                        nc.tensor.matmul(
                            po[:],
                            wo_sb[0][:, 128 * m : 128 * (m + 1)],
                            h0[:],
                            start=True,
                            stop=False,
                        )
                        nc.tensor.matmul(
                            po[:],
                            wo_sb[1][:, 128 * m : 128 * (m + 1)],
                            h1[:],
                            start=False,
                            stop=True,
                        )
                        outT = work.tile(
                            [128, SPAN], BF, tag="outT", bufs=4, name=f"ot{J}_{m}"
                        )
                        if m % 2 == 0:
                            nc.vector.tensor_copy(outT[:], po[:])
                        else:
                            nc.scalar.copy(outT[:], po[:])
                        if J == NSPAN - 1:
                            # finer split at the very end: the tail is
                            # DMA-drain bound, engines are idle by then
                            engs[m % 2].dma_start(
                                out[128 * m : 128 * m + 64, q0 : q0 + SPAN],
                                outT[0:64, :],
                            )
                            engs[(m + 1) % 2].dma_start(
                                out[128 * m + 64 : 128 * (m + 1), q0 : q0 + SPAN],
                                outT[64:128, :],
                            )
                        else:
                            nc.sync.dma_start(
                                out[128 * m : 128 * (m + 1), q0 : q0 + SPAN], outT[:]
                            )

                prev = None
                for J in range(NSPAN):
                    h0 = pair_attention(J, 0)
                    if prev is not None:
                        emit_oproj(*prev)
                    h1 = pair_attention(J, 1)
                    prev = (J, h0, h1)
                emit_oproj(*prev)

    nc.finalize()
    return nc


def _host_inputs(x, cos, sin, wq, wk, wv, wo):
    cosT = np.ascontiguousarray(cos.T).astype(np.float32)  # [64, S]
    sinT = np.ascontiguousarray(sin.T).astype(np.float32)
    s1n = np.concatenate([-sinT[0:32], sinT[32:64]], axis=0)  # [64, S]
    c2n = np.concatenate([cosT, cosT], axis=0).astype(BF16)  # [128, S]
    # partition-swapped: row p holds the sin factor for the partner row p^32,
    # so both DVE operands read from the same base partition
    s1w = np.concatenate([s1n[32:64], s1n[0:32]], axis=0)
    s2w = np.concatenate([s1w, s1w], axis=0).astype(BF16)
    cosT = cosT.astype(BF16)
    s1w = s1w.astype(BF16)
    # upper-triangular (incl diagonal) keep-mask for the causal boundary block
    dmaskh = (np.arange(128)[None, :] >= np.arange(128)[:, None]).astype(BF16)

    in_maps = []
    for c in range(N_CORES):
        b, g = c // 4, c % 4
        xT = np.ascontiguousarray(x[b].T).astype(BF16)
        wq_c = np.ascontiguousarray(wq[:, 256 * g : 256 * (g + 1)] / 8.0).astype(BF16)
        wkv_c = np.ascontiguousarray(
            np.concatenate(
                [wk[:, 64 * g : 64 * (g + 1)], wv[:, 64 * g : 64 * (g + 1)]], axis=1
            )
        ).astype(BF16)
        woR_c = np.ascontiguousarray(wo[256 * g : 256 * (g + 1), :]).astype(BF16)
        in_maps.append(
            {
                "xT": xT,
                "wq": wq_c,
                "wkv": wkv_c,
                "woR": woR_c,
                "c2": c2n,
                "s2": s2w,
                "c1": cosT,
                "s1": s1w,
                "dmask": dmaskh,
            }
        )
    return in_maps


def kernel(x, cos, sin, wq, wk, wv, wo):
    if "nc" not in _CACHED:
        _CACHED["nc"] = _build_nc()
    nc = _CACHED["nc"]
    in_maps = _host_inputs(
        np.asarray(x, np.float32),
        np.asarray(cos, np.float32),
        np.asarray(sin, np.float32),
        np.asarray(wq, np.float32),
        np.asarray(wk, np.float32),
        np.asarray(wv, np.float32),
        np.asarray(wo, np.float32),
    )
    res = run_bass_kernel_spmd(
        nc, in_maps, core_ids=list(range(N_CORES)), trace=TRACE
    )
    _CACHED["last_result"] = res
    out = np.empty((B, S, HID), dtype=np.float32)
    for b in range(B):
        acc = res.results[4 * b]["out"].astype(np.float32)
        for g in range(1, 4):
            acc += res.results[4 * b + g]["out"].astype(np.float32)
        out[b] = acc.T
    return out
